# revision 1
# baseline (speedup 1.0000x reference)
"""Single-head attention (B=4, T=4096, D_IN=1024, D_HEAD=D_OUT=64) on 8 TRN2
NeuronCores.

Sharding: core c handles batch b = c//2 and query-half h = c%2 (2048 queries),
computing K/V for the full sequence of its batch redundantly on both cores of
a pair.  Inputs are pre-transposed/permuted on the host so the device program
is identical on every core (SPMD):

  xt[c]  = x[b].T  with columns permuted so the core's own query-half comes
           first.  The s-permutation of K/V is harmless (softmax + weighted
           sum are permutation-invariant); queries come out in natural order.

Device pipeline per core (all matmuls on TensorE in fp32r rounding mode):
  A. qT = Wq.T @ x.T (own half),  [kT; vT] = [Wk|Wv].T @ x.T (full seq)
  B. v_aug[s,0:64] = v (PE-transpose of vT), v_aug[s,64] = 1.0
  C. per query-chunk: scoresT[s,tq] = kT(s-chunk) x qT;  ACT exp(scale*x)
     PSUM->SBUF;  out_augT[o,tq] += v_aug.T @ expT  (row 64 = softmax denom)
  D. PE-transpose out_augT, scale rows by reciprocal of denom, DMA out.
"""

import numpy as np

import concourse.bacc as bacc
import concourse.bass as bass
import concourse.mybir as mybir
import concourse.tile as tile
from concourse.bass_utils import run_bass_kernel_spmd

B, T, D_IN, D_HEAD, D_OUT = 4, 4096, 1024, 64, 64
N_CORES = 8
TQ = T // 2          # queries per core
ND = D_IN // 128     # contraction chunks (8)
NS = T // 128        # key/value chunks of 128 (32)
SCALE = float(1.0 / np.sqrt(np.float32(D_HEAD)))

F32 = mybir.dt.float32
F32R = mybir.dt.float32r
EXPF = mybir.ActivationFunctionType.Exp


def emit_body(nc, tc, io, dt_mm, phases="ABCD", n_iters=None):
    """Emit the per-core kernel body. io: dict of DRAM APs.

    Single scheduling window: projections for the second sequence half (and
    their DMAs) overlap the ACT-bound attention over the first half.  PSUM
    budget (8 banks): pa(2x1) + ps(2x2) + po(1x2) = 8; phase-B transposes and
    phase-D transposes borrow the 'pa'/'ps' slots respectively.
    """
    xt_d, wkv_d, wq_d = io["xt"], io["wkv"], io["wq"]
    bkv_d, bq_d, id_d, out_d = io["bkv"], io["bq"], io["ident"], io["out"]

    with (
        tc.tile_pool(name="const", bufs=1) as cpool,
        tc.tile_pool(name="xt", bufs=6) as xpool,
        tc.tile_pool(name="proj", bufs=1) as ppool,
        tc.tile_pool(name="exp", bufs=2) as epool,
        tc.tile_pool(name="outp", bufs=1) as opool,
        tc.tile_pool(name="psum", bufs=1, space="PSUM") as qpool,
    ):
        # ---- constants ----
        wkv_sb = cpool.tile([128, ND, 128], dt_mm)
        wq_sb = cpool.tile([128, ND, 64], dt_mm)
        bkv_sb = cpool.tile([128, 1], F32)
        bq_sb = cpool.tile([64, 1], F32)
        id_sb = cpool.tile([128, 128], dt_mm)
        nc.scalar.dma_start(wq_sb[:], wq_d.rearrange("(c p) h -> p c h", p=128))
        nc.scalar.dma_start(wkv_sb[:], wkv_d.rearrange("(c p) h -> p c h", p=128))
        nc.gpsimd.dma_start(bkv_sb[:], bkv_d[:])
        nc.gpsimd.dma_start(bq_sb[:], bq_d[:])
        nc.gpsimd.dma_start(id_sb[:], id_d[:])

        # per-pass tiles so consumers depend on exactly one producer each
        kvs = [ppool.tile([128, 512], dt_mm, name=f"kvs{i}") for i in range(8)]
        qts = [ppool.tile([64, 512], dt_mm, name=f"qts{i}") for i in range(4)]
        vau = [ppool.tile([128, 65], dt_mm, name=f"vau{i}") for i in range(NS)]
        osb = opool.tile([128, TQ // 128, 64], F32)
        if "D" not in phases:
            nc.vector.memset(osb[:], 0.0)

        def body():
            for s in range(NS):
                nc.gpsimd.memset(vau[s][:, 64:65].bitcast(F32), 1.0)

            # ---- phase A+B: projections, one PSUM bank per accumulation ----
            # One 2MB DMA per (half, t2) pass, alternating HWDGE rings.
            xt_tiles = {}

            def load_tile(half, t2):
                xt_t = xpool.tile([128, ND, 512], dt_mm, tag="xt",
                                  name=f"xt{half}_{t2}")
                src = xt_d[:, half * 2048 + t2 * 512:
                           half * 2048 + (t2 + 1) * 512]
                srcr = src.rearrange("(c p) t -> p c t", p=128)
                eng = nc.sync if (half * 4 + t2) % 2 == 0 else nc.scalar
                eng.dma_start(xt_t[:, 0:ND // 2, :], srcr[:, 0:ND // 2, :])
                eng.dma_start(xt_t[:, ND // 2:ND, :], srcr[:, ND // 2:ND, :])
                xt_tiles[(half, t2)] = xt_t

            def q_pass(t2):
                pq = qpool.tile([64, 512], F32, tag="pa", bufs=2, name=f"pq{t2}")
                for d in range(ND):
                    nc.tensor.matmul(pq[:], wq_sb[:, d, :],
                                     xt_tiles[(0, t2)][:, d, :],
                                     start=(d == 0), stop=(d == ND - 1))
                nc.vector.tensor_scalar_add(qts[t2][:], pq[:], bq_sb[:])

            def kv_pass(half, t2):
                pkv = qpool.tile([128, 512], F32, tag="pa", bufs=2,
                                 name=f"pkv{half}_{t2}")
                for d in range(ND):
                    nc.tensor.matmul(pkv[:], wkv_sb[:, d, :],
                                     xt_tiles[(half, t2)][:, d, :],
                                     start=(d == 0), stop=(d == ND - 1))
                i = half * 4 + t2
                nc.vector.tensor_scalar_add(kvs[i][:], pkv[:], bkv_sb[:])
                if "B" in phases:
                    for cc in range(4):
                        c = i * 4 + cc
                        pvt = qpool.tile([128, 64], dt_mm, tag="pa", bufs=2,
                                         name=f"pvt{c}")
                        nc.tensor.transpose(
                            pvt[:], kvs[i][64:128, cc * 128:(cc + 1) * 128],
                            id_sb[64:128, 64:128])
                        nc.vector.tensor_copy(vau[c][:, 0:64], pvt[:])

            pos = {}

            def attn_group(tqc, g):
                # 4 consecutive s-chunks of the attention pipeline for query
                # chunk tqc, emitted as soon as their K/V chunks exist.
                if "C" not in phases:
                    return
                if tqc not in pos:
                    pos[tqc] = qpool.tile([65, 1024], F32, tag="po", bufs=1,
                                          name=f"po{tqc}")
                po = pos[tqc]
                for s in range(4 * g, 4 * g + 4):
                    ps_t = qpool.tile([128, 1024], F32, tag="ps", bufs=2,
                                      name=f"ps{tqc}_{s}")
                    for j in range(2):
                        nc.tensor.matmul(
                            ps_t[:, j * 512:(j + 1) * 512],
                            kvs[s // 4][0:64, (s % 4) * 128:(s % 4 + 1) * 128],
                            qts[2 * tqc + j][:],
                            start=True, stop=True)
                    et = epool.tile([128, 1024], dt_mm, tag="et",
                                    name=f"et{tqc}_{s}")
                    nc.scalar.activation(et[:], ps_t[:], EXPF, scale=SCALE)
                    for j in range(2):
                        nc.tensor.matmul(
                            po[:, j * 512:(j + 1) * 512],
                            vau[s][:],
                            et[:, j * 512:(j + 1) * 512],
                            start=(s == 0), stop=(s == NS - 1))

            def finish(tqc):
                if "C" not in phases or "D" not in phases:
                    return
                oT = opool.tile([65, 1024], F32, tag="oT", bufs=2, name=f"oT{tqc}")
                nc.vector.tensor_copy(oT[:], pos[tqc][:])
                for j in range(8):
                    jj = tqc * 8 + j
                    pt = qpool.tile([128, 65], F32, tag="pa", bufs=2, name=f"pt{jj}")
                    nc.tensor.transpose(pt[:], oT[:, j * 128:(j + 1) * 128],
                                        id_sb[0:65, 0:65].bitcast(F32))
                    rec = opool.tile([128, 1], F32, tag="rec", bufs=2,
                                     name=f"rec{jj}")
                    nc.vector.reciprocal(rec[:], pt[:, 64:65])
                    nc.vector.tensor_scalar_mul(osb[:, jj, :], pt[:, 0:64], rec[:])
                odst = out_d.rearrange("(j p) o -> p j o", p=128)
                nc.sync.dma_start(odst[:, tqc * 8:(tqc + 1) * 8, :],
                                  osb[:, tqc * 8:(tqc + 1) * 8, :])

            for t2 in range(4):
                load_tile(0, t2)
            for t2 in range(4):
                load_tile(1, t2)
            # interleave attention (tqc=0) with K/V production so ACT starts
            # as soon as the first K/V chunks and qT[0:1024] exist
            q_pass(0)
            kv_pass(0, 0)
            q_pass(1)
            kv_pass(0, 1)
            attn_group(0, 0)
            q_pass(2)
            kv_pass(0, 2)
            attn_group(0, 1)
            q_pass(3)
            kv_pass(0, 3)
            attn_group(0, 2)
            for t2 in range(4):
                kv_pass(1, t2)
                attn_group(0, 3 + t2)
            attn_group(0, 7)
            for g in range(8):
                attn_group(1, g)
            finish(0)
            finish(1)

        if n_iters is None:
            body()
        else:
            with tc.For_i(0, n_iters, 1) as _i:
                body()


def build_program(dt_mm=F32R, phases="ABCD", n_iters=None):
    nc = bacc.Bacc("TRN2", target_bir_lowering=False, debug=False,
                   num_devices=N_CORES)
    io = {
        "xt": nc.dram_tensor("xt", [D_IN, T], dt_mm, kind="ExternalInput").ap(),
        "wkv": nc.dram_tensor("wkv", [D_IN, 128], dt_mm, kind="ExternalInput").ap(),
        "wq": nc.dram_tensor("wq", [D_IN, 64], dt_mm, kind="ExternalInput").ap(),
        "bkv": nc.dram_tensor("bkv", [128, 1], F32, kind="ExternalInput").ap(),
        "bq": nc.dram_tensor("bq", [64, 1], F32, kind="ExternalInput").ap(),
        "ident": nc.dram_tensor("ident", [128, 128], dt_mm, kind="ExternalInput").ap(),
        "out": nc.dram_tensor("out", [TQ, 64], F32, kind="ExternalOutput").ap(),
    }
    with tile.TileContext(nc) as tc:
        emit_body(nc, tc, io, dt_mm, phases=phases, n_iters=n_iters)
    nc.compile()
    return nc


_PROGRAM_CACHE = {}


def get_program(dt_mm=F32R):
    key = str(dt_mm)
    if key not in _PROGRAM_CACHE:
        _PROGRAM_CACHE[key] = build_program(dt_mm)
    return _PROGRAM_CACHE[key]


def make_in_maps(x, Wk, bk, Wq, bq, Wv, bv):
    x = np.asarray(x, dtype=np.float32)
    wkv = np.ascontiguousarray(np.concatenate([Wk, Wv], axis=1), dtype=np.float32)
    wq = np.ascontiguousarray(Wq, dtype=np.float32)
    bkv = np.concatenate([bk, bv]).astype(np.float32).reshape(128, 1)
    bqv = np.asarray(bq, dtype=np.float32).reshape(64, 1)
    ident = np.eye(128, dtype=np.float32)
    in_maps = []
    for c in range(N_CORES):
        b, half = c // 2, c % 2
        xb = x[b]
        own = xb[half * TQ:(half + 1) * TQ].T
        other = xb[(1 - half) * TQ:(2 - half) * TQ].T
        xt = np.ascontiguousarray(np.concatenate([own, other], axis=1))
        in_maps.append({"xt": xt, "wkv": wkv, "wq": wq, "bkv": bkv,
                        "bq": bqv, "ident": ident})
    return in_maps


def assemble(results):
    out = np.empty((B, T, D_OUT), dtype=np.float32)
    for c in range(N_CORES):
        b, half = c // 2, c % 2
        out[b, half * TQ:(half + 1) * TQ, :] = results[c]["out"]
    return out


def kernel(x, Wk, bk, Wq, bq, Wv, bv):
    nc = get_program()
    in_maps = make_in_maps(x, Wk, bk, Wq, bq, Wv, bv)
    res = run_bass_kernel_spmd(nc, in_maps, list(range(N_CORES)))
    return assemble(res.results)



# revision 14
# speedup vs baseline: 1.3702x; 1.3702x over previous
"""Single-head attention (B=4, T=4096, D_IN=1024, D_HEAD=D_OUT=64) on 8 TRN2
NeuronCores.

Sharding: core c handles batch b = c//2 and query-half h = c%2 (2048 queries),
computing K/V for the full sequence of its batch redundantly on both cores of
a pair.  x is transposed/permuted AND cast to bf16 on the host (own query-half
columns first), halving HBM traffic.

Device pipeline per core (all matmul operands bf16, 1 PE-cycle/row):
  A. Projections: own-half passes use [Wk|Wq] packed -> psum[128,512]
     (k rows 0:64 evict partition-direct to kt, q rows 64:128 staged and
     SBUF->SBUF DMA'd down to qt partitions 0:64); other-half passes Wk only.
     V is computed directly in [s, o] orientation (lhsT = x-chunk, rhs = Wv),
     eliminating all PE transposes.
  B. Scores: out[s 128, t 512] per mm into psum window [128, 1024];
     ACT exp(scale*x) -> et bf16 [128, 1024].
  C. AV reoriented: po[t 128, o 64] += et_chunk.T @ v[s, o] (free dim 64);
     denominator pd[t, 1] += et_chunk.T @ ones.  8 po chunks + pd packed
     per psum bank (first-touch start=True, last-touch stop=True; psum
     written-bit semantics make multi-region banks exact).
  D. osb = po * reciprocal(pd) (one DVE reciprocal + one broadcast multiply
     per query half), DMA out.  v-bias added host-side (softmax weights sum
     to 1, so +bv is exact).
"""

import numpy as np
import ml_dtypes

import concourse.bacc as bacc
import concourse.bass as bass
import concourse.mybir as mybir
import concourse.tile as tile
from concourse.bass_utils import run_bass_kernel_spmd

B, T, D_IN, D_HEAD, D_OUT = 4, 4096, 1024, 64, 64
N_CORES = 8
TQ = T // 2          # queries per core
ND = D_IN // 128     # contraction chunks (8)
NS = T // 128        # key chunks of 128 (32)
NP = T // 512        # x passes (8); passes 0-3 are the own query half
SCALE = float(1.0 / np.sqrt(np.float32(D_HEAD)))

F32 = mybir.dt.float32
F32R = mybir.dt.float32r
BF16 = mybir.dt.bfloat16
EXPF = mybir.ActivationFunctionType.Exp


def emit_body(nc, tc, io, dt_mm=None, phases="ABCD", n_iters=None):
    """Emit the per-core kernel body. io: dict of DRAM APs."""
    xt_d, wall_d, id_d = io["xt"], io["wall"], io["ident"]
    bkq_d, bkk_d, out_d = io["bkq"], io["bkk"], io["out"]

    with (
        tc.tile_pool(name="const", bufs=1) as cpool,
        tc.tile_pool(name="xt", bufs=4) as xpool,
        tc.tile_pool(name="proj", bufs=1) as ppool,
        tc.tile_pool(name="stg", bufs=2) as spool,
        tc.tile_pool(name="exp", bufs=28) as epool,
        tc.tile_pool(name="outp", bufs=1) as opool,
        tc.tile_pool(name="psum", bufs=1, space="PSUM") as qpool,
    ):
        # ---- constants: issue on the SP ring ahead of the xt streams so the
        # (serialized) DMA engines deliver them first ----
        wall_sb = cpool.tile([128, ND, 256], BF16)
        id_sb = cpool.tile([128, 64], BF16)
        bkq_sb = cpool.tile([128, 1], F32)
        bkk_sb = cpool.tile([64, 1], F32)
        ones_sb = cpool.tile([128, 1], BF16)
        nc.sync.dma_start(wall_sb[:], wall_d[:])
        nc.sync.dma_start(id_sb[:], id_d[:])
        nc.gpsimd.dma_start(bkq_sb[:], bkq_d[:])
        nc.gpsimd.dma_start(bkk_sb[:], bkk_d[:])
        nc.gpsimd.memset(ones_sb[:], 1.0)
        wkq_sb = wall_sb[:, :, 0:128]
        wk_sb = wall_sb[:, :, 128:192]
        wv_sb = wall_sb[:, :, 192:256]

        # persistent per-iteration tensors
        kt = ppool.tile([64, T], BF16)             # k[h, s]
        qt = ppool.tile([64, TQ], BF16)            # q[h, t]
        v_sb = ppool.tile([128, NS, 64], BF16)     # v[s, o]
        osb = opool.tile([128, 16, 64], F32)

        def body():
            xt_tiles = {}

            def load_xt(p):
                xt_t = xpool.tile([128, ND, 512], BF16, tag="xt", name=f"xt{p}")
                src = xt_d[:, p * 512:(p + 1) * 512]
                srcr = src.rearrange("(c p) t -> p c t", p=128)
                nc.sync.dma_start(xt_t[:, 0:ND // 2, :], srcr[:, 0:ND // 2, :])
                nc.sync.dma_start(xt_t[:, ND // 2:ND, :], srcr[:, ND // 2:ND, :])
                xt_tiles[p] = xt_t

            def own_pass(p):
                # k rows 0:64, q rows 64:128
                pkq = qpool.tile([128, 512], F32, tag="pa", bufs=2, name=f"pkq{p}")
                for d in range(ND):
                    nc.tensor.matmul(pkq[:], wkq_sb[:, d, :], xt_tiles[p][:, d, :],
                                     start=(d == 0), stop=(d == ND - 1))
                cols = slice(p * 512, (p + 1) * 512)
                stg = spool.tile([128, 512], BF16, tag="stg", name=f"stg{p}")
                nc.vector.tensor_scalar_add(kt[:, cols], pkq[0:64, :],
                                            bkq_sb[0:64])
                nc.vector.tensor_scalar_add(stg[64:128, :], pkq[64:128, :],
                                            bkq_sb[64:128])
                return stg

            def q_fix(p, stg):
                # shift q rows from partitions 64:128 down to 0:64 on the PE
                # (identity matmul) - avoids a DMA stuck behind the xt streams
                pqf = qpool.tile([64, 512], F32, tag="pa", bufs=2,
                                 name=f"pqf{p}")
                nc.tensor.matmul(pqf[:], id_sb[64:128, :], stg[64:128, :],
                                 start=True, stop=True)
                nc.vector.tensor_copy(qt[:, p * 512:(p + 1) * 512], pqf[:])

            def k_pass(p):
                pk = qpool.tile([64, 512], F32, tag="pa", bufs=2, name=f"pk{p}")
                for d in range(ND):
                    nc.tensor.matmul(pk[:], wk_sb[:, d, :], xt_tiles[p][:, d, :],
                                     start=(d == 0), stop=(d == ND - 1))
                cols = slice(p * 512, (p + 1) * 512)
                nc.vector.tensor_scalar_add(kt[:, cols], pk[:], bkk_sb[:])

            def v_pass(p):
                pv = qpool.tile([128, 4, 64], F32, tag="pa", bufs=2, name=f"pv{p}")
                for sc in range(4):
                    for d in range(ND):
                        nc.tensor.matmul(
                            pv[:, sc, :],
                            xt_tiles[p][:, d, sc * 128:(sc + 1) * 128],
                            wv_sb[:, d, :],
                            start=(sc == 0 and d == 0),
                            stop=(sc == 3 and d == ND - 1))
                nc.vector.tensor_copy(v_sb[:, p * 4:(p + 1) * 4, :], pv[:])

            pos = {}
            pds = {}
            ets = {}

            def gp_activation(out, in_, func, scale):
                # raw InstActivation on the (otherwise idle) GPSIMD engine
                eng = nc.gpsimd
                bias = nc.const_aps.scalar_like(0.0, in_)
                ins = [eng.lower_ap(in_), eng.lower_ap(bias),
                       mybir.ImmediateValue(dtype=mybir.dt.float32, value=scale),
                       mybir.ImmediateValue(dtype=mybir.dt.float32, value=0.0)]
                return eng.add_instruction(mybir.InstActivation(
                    name=nc.get_next_instruction_name(), func=func,
                    ins=ins, outs=[eng.lower_ap(out)]))

            def pd_mms(w, s):
                if w not in pds:
                    pds[w] = qpool.tile([128, 8], F32, tag="pa", bufs=2,
                                        name=f"pd{w}")
                pd = pds[w]
                for tc in range(8):
                    nc.tensor.matmul(pd[:, tc:tc + 1],
                                     ets[(w, s)][:, tc * 128:(tc + 1) * 128],
                                     ones_sb[:],
                                     start=(s == 0 and tc == 0),
                                     stop=(s == NS - 1 and tc == 7))

            pending = []

            def av_mms(w, s, inline_pd):
                po = pos[w]
                et = ets[(w, s)]
                for tc in range(8):
                    nc.tensor.matmul(po[:, tc, :],
                                     et[:, tc * 128:(tc + 1) * 128],
                                     v_sb[:, s, :],
                                     start=(s == 0 and tc == 0),
                                     stop=(s == NS - 1 and tc == 7))
                if inline_pd:
                    pd_mms(w, s)

            def flush_pending():
                while pending:
                    av_mms(*pending.pop(0))

            def win(w, s, inline_pd=False):
                ps = qpool.tile([128, 1024], F32, tag="ps", bufs=2,
                                name=f"ps{w}_{s}")
                for j in range(2):
                    nc.tensor.matmul(
                        ps[:, j * 512:(j + 1) * 512],
                        kt[:, s * 128:(s + 1) * 128],
                        qt[:, w * 1024 + j * 512:w * 1024 + (j + 1) * 512],
                        start=True, stop=True)
                et = epool.tile([128, 1024], BF16, tag="et", name=f"et{w}_{s}")
                nc.scalar.activation(et[:], ps[:], EXPF, scale=SCALE)
                ets[(w, s)] = et
                if w not in pos:
                    pos[w] = qpool.tile([128, 8, 64], F32, tag=f"po{w}", bufs=1,
                                        name=f"po{w}")
                # defer this window's AV until after the next window's scores,
                # so the activation latency is off the PE's in-order path
                flush_pending()
                pending.append((w, s, inline_pd))

            def finish(w):
                pd, po = pds[w], pos[w]
                rec = opool.tile([128, 8], F32, tag="rec", bufs=2,
                                 name=f"rec{w}")
                nc.vector.reciprocal(rec[:], pd[:, 0:8])
                nc.vector.tensor_mul(osb[:, w * 8:(w + 1) * 8, :], po[:],
                                     rec[:, :, None].broadcast_to([128, 8, 64]))
                odst = out_d.rearrange("(j p) o -> p j o", p=128)
                nc.sync.dma_start(odst[:, w * 8:(w + 1) * 8, :],
                                  osb[:, w * 8:(w + 1) * 8, :])

            # ---- emission schedule ----
            load_xt(0)
            load_xt(1)
            stg0 = own_pass(0)
            v_pass(0)
            q_fix(0, stg0)
            stg1 = own_pass(1)
            v_pass(1)
            q_fix(1, stg1)
            for s in range(0, 4):
                win(0, s)
            # passes 2-7 interleaved with w=0 windows s=4..27
            for p in range(2, NP):
                load_xt(p)
                if p < 4:
                    stgp = own_pass(p)
                    v_pass(p)
                    q_fix(p, stgp)
                else:
                    k_pass(p)
                    v_pass(p)
                for s in range(4 * (p - 2) + 4, 4 * (p - 2) + 8):
                    win(0, s)
            # all proj psum tiles emitted; pd0 can now take a "pa" slot
            for s in range(0, 28):
                pd_mms(0, s)
            for s in range(28, NS):
                win(0, s, inline_pd=True)
            flush_pending()
            finish(0)
            for s in range(NS):
                win(1, s, inline_pd=True)
            flush_pending()
            finish(1)

        if n_iters is None:
            body()
        else:
            with tc.For_i(0, n_iters, 1) as _i:
                body()


def build_program(dt_mm=None, phases="ABCD", n_iters=None):
    nc = bacc.Bacc("TRN2", target_bir_lowering=False, debug=False,
                   num_devices=N_CORES)
    io = {
        "xt": nc.dram_tensor("xt", [D_IN, T], BF16, kind="ExternalInput").ap(),
        "wall": nc.dram_tensor("wall", [128, ND, 256], BF16, kind="ExternalInput").ap(),
        "ident": nc.dram_tensor("ident", [128, 64], BF16, kind="ExternalInput").ap(),
        "bkq": nc.dram_tensor("bkq", [128, 1], F32, kind="ExternalInput").ap(),
        "bkk": nc.dram_tensor("bkk", [64, 1], F32, kind="ExternalInput").ap(),
        "out": nc.dram_tensor("out", [TQ, D_OUT], F32, kind="ExternalOutput").ap(),
    }
    with tile.TileContext(nc) as tc:
        emit_body(nc, tc, io, dt_mm, phases=phases, n_iters=n_iters)
    nc.compile()
    return nc


_PROGRAM_CACHE = {}


def get_program(dt_mm=None):
    key = str(dt_mm)
    if key not in _PROGRAM_CACHE:
        _PROGRAM_CACHE[key] = build_program(dt_mm)
    return _PROGRAM_CACHE[key]


def make_in_maps(x, Wk, bk, Wq, bq, Wv, bv):
    bf = ml_dtypes.bfloat16
    x = np.asarray(x, dtype=np.float32)
    # pack [d, m] -> [p, c, m] with d = c*128 + p; m-order: kq(128)|k(64)|v(64)
    wcat = np.concatenate([Wk, Wq, Wk, Wv], axis=1).astype(np.float32)
    wall = np.ascontiguousarray(
        wcat.reshape(ND, 128, 256).transpose(1, 0, 2)).astype(bf)
    ident = np.zeros((128, 64), dtype=np.float32)
    ident[64:128, :] = np.eye(64)
    ident = ident.astype(bf)
    bkq = np.concatenate([bk, bq]).astype(np.float32).reshape(128, 1)
    bkk = np.asarray(bk, dtype=np.float32).reshape(64, 1)
    in_maps = []
    for c in range(N_CORES):
        b, half = c // 2, c % 2
        xb = x[b]
        own = xb[half * TQ:(half + 1) * TQ].T
        other = xb[(1 - half) * TQ:(2 - half) * TQ].T
        xt = np.ascontiguousarray(
            np.concatenate([own, other], axis=1)).astype(bf)
        in_maps.append({"xt": xt, "wall": wall, "ident": ident,
                       "bkq": bkq, "bkk": bkk})
    return in_maps


def assemble(results, bv):
    out = np.empty((B, T, D_OUT), dtype=np.float32)
    bv = np.asarray(bv, dtype=np.float32)
    for c in range(N_CORES):
        b, half = c // 2, c % 2
        out[b, half * TQ:(half + 1) * TQ, :] = results[c]["out"] + bv
    return out


def kernel(x, Wk, bk, Wq, bq, Wv, bv):
    nc = get_program()
    in_maps = make_in_maps(x, Wk, bk, Wq, bq, Wv, bv)
    res = run_bass_kernel_spmd(nc, in_maps, list(range(N_CORES)))
    return assemble(res.results, bv)


# revision 37
# speedup vs baseline: 1.6324x; 1.1914x over previous
"""Single-head attention (B=4, T=4096, D_IN=1024, D_HEAD=D_OUT=64) on 8 TRN2
NeuronCores.

Sharding: core c handles batch b = c//2 and query-half h = c%2 (2048 queries),
computing K/V for the full sequence of its batch redundantly on both cores of
a pair.  x is transposed/permuted AND cast to bf16 on the host (own query-half
columns first), halving HBM traffic.

Device pipeline per core (all matmul operands bf16, 1 PE-cycle/row):
  A. Projections: own-half passes use [Wk|Wq] packed -> psum[128,512]
     (k rows 0:64 evict partition-direct to kt, q rows 64:128 staged and
     SBUF->SBUF DMA'd down to qt partitions 0:64); other-half passes Wk only.
     V is computed directly in [s, o] orientation (lhsT = x-chunk, rhs = Wv),
     eliminating all PE transposes.
  B. Scores: out[s 128, t 512] per mm into psum window [128, 1024];
     ACT exp(scale*x) -> et bf16 [128, 1024].
  C. AV reoriented: po[t 128, o 64] += et_chunk.T @ v[s, o] (free dim 64);
     denominator pd[t, 1] += et_chunk.T @ ones.  8 po chunks + pd packed
     per psum bank (first-touch start=True, last-touch stop=True; psum
     written-bit semantics make multi-region banks exact).
  D. osb = po * reciprocal(pd) (one DVE reciprocal + one broadcast multiply
     per query half), DMA out.  v-bias added host-side (softmax weights sum
     to 1, so +bv is exact).
"""

import numpy as np
import ml_dtypes

import concourse.bacc as bacc
import concourse.bass as bass
import concourse.mybir as mybir
import concourse.tile as tile
from concourse.bass_utils import run_bass_kernel_spmd

B, T, D_IN, D_HEAD, D_OUT = 4, 4096, 1024, 64, 64
N_CORES = 8
TQ = T // 2          # queries per core
ND = D_IN // 128     # contraction chunks (8)
NS = T // 128        # key chunks of 128 (32)
NP = T // 512        # x passes (8); passes 0-3 are the own query half
SCALE = float(1.0 / np.sqrt(np.float32(D_HEAD)))
LOG2E = 1.4426950408889634
EXP_A = SCALE * LOG2E * 128.0          # folds softmax scale into Schraudolph
EXP_B = 127.0 * 128.0 - 7.42

F32 = mybir.dt.float32
I16 = mybir.dt.int16
F32R = mybir.dt.float32r
BF16 = mybir.dt.bfloat16
EXPF = mybir.ActivationFunctionType.Exp
EXP_PATTERN = "AD"


def emit_body(nc, tc, io, dt_mm=None, phases="ABCD", n_iters=None):
    """Emit the per-core kernel body. io: dict of DRAM APs."""
    xt_d, wall_d, id_d = io["xt"], io["wall"], io["ident"]
    bkq_d, bkk_d, out_d = io["bkq"], io["bkk"], io["out"]

    with (
        tc.tile_pool(name="const", bufs=1) as cpool,
        tc.tile_pool(name="xt", bufs=6) as xpool,
        tc.tile_pool(name="proj", bufs=1) as ppool,
        tc.tile_pool(name="stg", bufs=2) as spool,
        tc.tile_pool(name="exp", bufs=28) as epool,
        tc.tile_pool(name="outp", bufs=1) as opool,
        tc.tile_pool(name="psum", bufs=1, space="PSUM") as qpool,
    ):
        # ---- constants: issue on the SP ring ahead of the xt streams so the
        # (serialized) DMA engines deliver them first ----
        wall_sb = cpool.tile([128, ND * 256], BF16)
        id_sb = cpool.tile([128, 64], BF16)
        bkq_sb = cpool.tile([128, 1], F32)
        bkk_sb = cpool.tile([64, 1], F32)
        ones_sb = cpool.tile([128, 1], BF16)
        nc.sync.dma_start(wall_sb[:, 0:1024], wall_d[:, 0:1024])
        nc.gpsimd.dma_start(bkq_sb[:], bkq_d[:])
        nc.gpsimd.dma_start(bkk_sb[:], bkk_d[:])
        nc.gpsimd.memset(ones_sb[:], 1.0)
        wkq_sb = wall_sb[:, 0:1024].rearrange("p (c m) -> p c m", m=128)
        wk_sb = wall_sb[:, 1024:1536].rearrange("p (c m) -> p c m", m=64)
        wv_sb = wall_sb[:, 1536:2048].rearrange("p (c m) -> p c m", m=64)

        # persistent per-iteration tensors
        kt = ppool.tile([64, T], BF16)             # k[h, s]
        qt = ppool.tile([64, TQ], BF16)            # q[h, t]
        v_sb = ppool.tile([128, NS, 64], BF16)     # v[s, o]
        pd_sb = ppool.tile([128, 2, 4, 8], F32)    # per-octet denom partials
        osb = opool.tile([128, 16, 64], F32)

        def body():
            xt_tiles = {}

            def load_xt(p):
                xt_t = xpool.tile([128, ND, 512], BF16, tag="xt", name=f"xt{p}")
                src = xt_d[:, p * 512:(p + 1) * 512]
                srcr = src.rearrange("(c p) t -> p c t", p=128)
                nc.sync.dma_start(xt_t[:, 0:ND // 2, :], srcr[:, 0:ND // 2, :])
                nc.sync.dma_start(xt_t[:, ND // 2:ND, :], srcr[:, ND // 2:ND, :])
                xt_tiles[p] = xt_t

            def own_pass(p):
                # k rows 0:64, q rows 64:128
                pkq = qpool.tile([128, 512], F32, tag="pa", bufs=2, name=f"pkq{p}")
                for d in range(ND):
                    nc.tensor.matmul(pkq[:], wkq_sb[:, d, :], xt_tiles[p][:, d, :],
                                     start=(d == 0), stop=(d == ND - 1))
                cols = slice(p * 512, (p + 1) * 512)
                stg = spool.tile([128, 512], BF16, tag="stg", name=f"stg{p}")
                nc.vector.tensor_scalar_add(kt[:, cols], pkq[0:64, :],
                                            bkq_sb[0:64])
                nc.vector.tensor_scalar_add(stg[64:128, :], pkq[64:128, :],
                                            bkq_sb[64:128])
                return stg

            def q_fix(p, stg):
                # shift q rows from partitions 64:128 down to 0:64 on the PE
                # (identity matmul) - avoids a DMA stuck behind the xt streams
                pqf = qpool.tile([64, 512], F32, tag="pa", bufs=2,
                                 name=f"pqf{p}")
                nc.tensor.matmul(pqf[:], id_sb[64:128, :], stg[64:128, :],
                                 start=True, stop=True)
                nc.vector.tensor_copy(qt[:, p * 512:(p + 1) * 512], pqf[:])

            def k_pass(p):
                pk = qpool.tile([64, 512], F32, tag="pa", bufs=2, name=f"pk{p}")
                for d in range(ND):
                    nc.tensor.matmul(pk[:], wk_sb[:, d, :], xt_tiles[p][:, d, :],
                                     start=(d == 0), stop=(d == ND - 1))
                cols = slice(p * 512, (p + 1) * 512)
                nc.vector.tensor_scalar_add(kt[:, cols], pk[:], bkk_sb[:])

            def v_pass(p):
                pv = qpool.tile([128, 4, 64], F32, tag="pa", bufs=2, name=f"pv{p}")
                for sc in range(4):
                    for d in range(ND):
                        nc.tensor.matmul(
                            pv[:, sc, :],
                            xt_tiles[p][:, d, sc * 128:(sc + 1) * 128],
                            wv_sb[:, d, :],
                            start=(sc == 0 and d == 0),
                            stop=(sc == 3 and d == ND - 1))
                nc.vector.tensor_copy(v_sb[:, p * 4:(p + 1) * 4, :], pv[:])

            pos = {}
            pds = {}
            ets = {}
            pd_done = {}
            pdps = {}


            def pd_octet(w, g):
                # denominator partial for windows s = 8g..8g+7: short-lived
                # psum tile rotating through the "pa" slots, evicted to SBUF
                pdp = qpool.tile([128, 8], F32, tag="pa", bufs=2,
                                 name=f"pdp{w}_{g}")
                for i, s in enumerate(range(8 * g, 8 * g + 8)):
                    for et, tc0, ntc in ets[(w, s)]:
                        for j in range(ntc):
                            tc = tc0 + j
                            nc.tensor.matmul(
                                pdp[:, tc:tc + 1],
                                et[:, j * 128:(j + 1) * 128],
                                ones_sb[:],
                                start=(i == 0 and tc == 0),
                                stop=(i == 7 and tc == 7))
                if g < 3:
                    nc.vector.tensor_copy(pd_sb[:, w, g, :], pdp[:])
                else:
                    pdps[w] = pdp
                pd_done[w] = pd_done.get(w, 0) + 1
                if pd_done[w] in (2, 3):
                    pd_presum(w)

            pending = []

            def av_mms(w, s, pi):
                po = pos[w]
                et, tc0, ntc = ets[(w, s)][pi]
                for j in range(ntc):
                    tc = tc0 + j
                    nc.tensor.matmul(po[:, tc, :],
                                     et[:, j * 128:(j + 1) * 128],
                                     v_sb[:, s, :],
                                     start=(s == 0 and tc == 0),
                                     stop=(s == NS - 1 and tc == 7))
                if s % 8 == 7 and tc0 + ntc == 8:
                    pd_octet(w, s // 8)

            def flush_pending():
                while pending:
                    av_mms(*pending.pop(0))

            def win(w, s, half=None):
                # half=None: full 1024-col window; half=0/1: 512-col window
                # covering query chunks tc 0..3 / 4..7 (used at startup while
                # only part of qt exists)
                ncols = 1024 if half is None else 512
                col0 = w * 1024 + (0 if half in (None, 0) else 512)
                tc0 = 0 if half in (None, 0) else 4
                ps = qpool.tile([128, ncols], F32, tag="ps", bufs=2,
                                name=f"ps{w}_{s}_{half}")
                for j in range(ncols // 512):
                    nc.tensor.matmul(
                        ps[:, j * 512:(j + 1) * 512],
                        kt[:, s * 128:(s + 1) * 128],
                        qt[:, col0 + j * 512:col0 + (j + 1) * 512],
                        start=True, stop=True)
                et = epool.tile([128, ncols], BF16, tag="et",
                                name=f"et{w}_{s}_{half}")
                idx = w * NS + s
                if half is None and idx % 3 == 2 and 6 < idx < 63:
                    gp_activation(et, ps, EXPF, SCALE)
                else:
                    nc.scalar.activation(et[:], ps[:], EXPF, scale=SCALE)
                ets.setdefault((w, s), []).append((et, tc0, ncols // 512 * 4))
                if w not in pos:
                    pos[w] = qpool.tile([128, 8, 64], F32, tag=f"po{w}", bufs=1,
                                        name=f"po{w}")
                # defer this window's AV until after the next window's scores,
                # so the activation latency is off the PE's in-order path
                flush_pending()
                pending.append((w, s, len(ets[(w, s)]) - 1))

            dsums = {}

            def pd_presum(w):
                # partial denominator sums emitted as octets complete
                dsum = dsums.setdefault(w, opool.tile(
                    [128, 2, 8], F32, tag="dsum", bufs=2, name=f"dsum{w}"))
                g = pd_done[w]
                if g == 2:
                    nc.vector.tensor_add(dsum[:, 0, :], pd_sb[:, w, 0, :],
                                         pd_sb[:, w, 1, :])
                elif g == 3:
                    nc.vector.tensor_add(dsum[:, 1, :], dsum[:, 0, :],
                                         pd_sb[:, w, 2, :])

            def finish(w):
                po = pos[w]
                dsum = dsums[w]
                rec = opool.tile([128, 8], F32, tag="rec", bufs=2,
                                 name=f"rec{w}")
                nc.vector.tensor_add(rec[:], dsum[:, 1, :], pdps[w][:])
                nc.vector.reciprocal(rec[:], rec[:])
                odst = out_d.rearrange("(j p) o -> p j o", p=128)
                for hh in range(2):
                    jj = w * 8 + hh * 4
                    nc.vector.tensor_mul(
                        osb[:, jj:jj + 4, :], po[:, hh * 4:(hh + 1) * 4, :],
                        rec[:, hh * 4:(hh + 1) * 4, None].broadcast_to(
                            [128, 4, 64]))
                    nc.sync.dma_start(odst[:, jj:jj + 4, :],
                                      osb[:, jj:jj + 4, :])

            # ---- emission schedule ----
            xt_tiles[0] = xpool.tile([128, ND, 512], BF16, tag="xt",
                                     name="xt0")
            xt_tiles[1] = xpool.tile([128, ND, 512], BF16, tag="xt",
                                     name="xt1")
            for p in range(2):
                srcr = xt_d[:, p * 512:(p + 1) * 512].rearrange(
                    "(c p) t -> p c t", p=128)
                for h in range(4):
                    nc.sync.dma_start(
                        xt_tiles[p][:, h * (ND // 4):(h + 1) * (ND // 4), :],
                        srcr[:, h * (ND // 4):(h + 1) * (ND // 4), :])
                if p == 0:
                    nc.sync.dma_start(wall_sb[:, 1024:2048],
                                      wall_d[:, 1024:2048])
                    nc.sync.dma_start(id_sb[:], id_d[:])
            stg0 = own_pass(0)
            v_pass(0)
            q_fix(0, stg0)
            # startup mini-windows: only qt cols 0:512 exist yet
            for s in range(0, 4):
                win(0, s, half=0)
            stg1 = own_pass(1)
            v_pass(1)
            q_fix(1, stg1)
            for s in range(0, 4):
                win(0, s, half=1)
            # passes 2-7 interleaved with windows; once qt is complete
            # (pass 3) also consume w=1 so windows never outrun production
            done1 = 0
            for p in range(2, NP):
                load_xt(p)
                if p < 4:
                    stgp = own_pass(p)
                    v_pass(p)
                    q_fix(p, stgp)
                else:
                    k_pass(p)
                    v_pass(p)
                for s in range(4 * (p - 2) + 4, 4 * (p - 2) + 8):
                    win(0, s)
                    if p >= 4:
                        win(1, done1)
                        done1 += 1
            for s in range(28, NS):
                win(0, s)
                win(1, done1)
                done1 += 1
            first_tail = done1
            for s in range(first_tail, NS):
                win(1, s)
                if s == first_tail + 1:
                    flush_w0 = [p_ for p_ in pending if p_[0] == 0]
                    for p_ in flush_w0:
                        pending.remove(p_)
                        av_mms(*p_)
                    finish(0)
            flush_pending()
            finish(1)

        if n_iters is None:
            body()
        else:
            with tc.For_i(0, n_iters, 1) as _i:
                body()


def build_program(dt_mm=None, phases="ABCD", n_iters=None):
    nc = bacc.Bacc("TRN2", target_bir_lowering=False, debug=False,
                   num_devices=N_CORES)
    io = {
        "xt": nc.dram_tensor("xt", [D_IN, T], BF16, kind="ExternalInput").ap(),
        "wall": nc.dram_tensor("wall", [128, ND * 256], BF16, kind="ExternalInput").ap(),
        "ident": nc.dram_tensor("ident", [128, 64], BF16, kind="ExternalInput").ap(),
        "bkq": nc.dram_tensor("bkq", [128, 1], F32, kind="ExternalInput").ap(),
        "bkk": nc.dram_tensor("bkk", [64, 1], F32, kind="ExternalInput").ap(),
        "out": nc.dram_tensor("out", [TQ, D_OUT], F32, kind="ExternalOutput").ap(),
    }
    with tile.TileContext(nc) as tc:
        emit_body(nc, tc, io, dt_mm, phases=phases, n_iters=n_iters)
    nc.compile()
    return nc


_PROGRAM_CACHE = {}


def get_program(dt_mm=None):
    key = str(dt_mm)
    if key not in _PROGRAM_CACHE:
        _PROGRAM_CACHE[key] = build_program(dt_mm)
    return _PROGRAM_CACHE[key]


def make_in_maps(x, Wk, bk, Wq, bq, Wv, bv):
    bf = ml_dtypes.bfloat16
    x = np.asarray(x, dtype=np.float32)
    # pack [p, cols]: wkq block (8 chunks x 128) | wk (8 x 64) | wv (8 x 64)
    def blk(Wm):
        m = Wm.shape[1]
        return Wm.astype(np.float32).reshape(ND, 128, m).transpose(1, 0, 2) \
            .reshape(128, ND * m)
    wall = np.ascontiguousarray(np.concatenate(
        [blk(np.concatenate([Wk, Wq], axis=1)), blk(Wk), blk(Wv)],
        axis=1)).astype(bf)
    ident = np.zeros((128, 64), dtype=np.float32)
    ident[64:128, :] = np.eye(64)
    ident = ident.astype(bf)
    bkq = np.concatenate([bk, bq]).astype(np.float32).reshape(128, 1)
    bkk = np.asarray(bk, dtype=np.float32).reshape(64, 1)
    in_maps = []
    for c in range(N_CORES):
        b, half = c // 2, c % 2
        xb = x[b]
        own = xb[half * TQ:(half + 1) * TQ].T
        other = xb[(1 - half) * TQ:(2 - half) * TQ].T
        xt = np.ascontiguousarray(
            np.concatenate([own, other], axis=1)).astype(bf)
        in_maps.append({"xt": xt, "wall": wall, "ident": ident,
                       "bkq": bkq, "bkk": bkk})
    return in_maps


def assemble(results, bv):
    out = np.empty((B, T, D_OUT), dtype=np.float32)
    bv = np.asarray(bv, dtype=np.float32)
    for c in range(N_CORES):
        b, half = c // 2, c % 2
        out[b, half * TQ:(half + 1) * TQ, :] = results[c]["out"] + bv
    return out


def kernel(x, Wk, bk, Wq, bq, Wv, bv):
    nc = get_program()
    in_maps = make_in_maps(x, Wk, bk, Wq, bq, Wv, bv)
    res = run_bass_kernel_spmd(nc, in_maps, list(range(N_CORES)))
    return assemble(res.results, bv)


# revision 38
# speedup vs baseline: 1.6365x; 1.0025x over previous
"""Single-head attention (B=4, T=4096, D_IN=1024, D_HEAD=D_OUT=64) on 8 TRN2
NeuronCores.

Sharding: core c handles batch b = c//2 and query-half h = c%2 (2048 queries),
computing K/V for the full sequence of its batch redundantly on both cores of
a pair.  x is transposed/permuted AND cast to bf16 on the host (own query-half
columns first), halving HBM traffic.

Device pipeline per core (all matmul operands bf16, 1 PE-cycle/row):
  A. Projections: own-half passes use [Wk|Wq] packed -> psum[128,512]
     (k rows 0:64 evict partition-direct to kt, q rows 64:128 staged and
     SBUF->SBUF DMA'd down to qt partitions 0:64); other-half passes Wk only.
     V is computed directly in [s, o] orientation (lhsT = x-chunk, rhs = Wv),
     eliminating all PE transposes.
  B. Scores: out[s 128, t 512] per mm into psum window [128, 1024];
     ACT exp(scale*x) -> et bf16 [128, 1024].
  C. AV reoriented: po[t 128, o 64] += et_chunk.T @ v[s, o] (free dim 64);
     denominator pd[t, 1] += et_chunk.T @ ones.  8 po chunks + pd packed
     per psum bank (first-touch start=True, last-touch stop=True; psum
     written-bit semantics make multi-region banks exact).
  D. osb = po * reciprocal(pd) (one DVE reciprocal + one broadcast multiply
     per query half), DMA out.  v-bias added host-side (softmax weights sum
     to 1, so +bv is exact).
"""

import numpy as np
import ml_dtypes

import concourse.bacc as bacc
import concourse.bass as bass
import concourse.mybir as mybir
import concourse.tile as tile
from concourse.bass_utils import run_bass_kernel_spmd

B, T, D_IN, D_HEAD, D_OUT = 4, 4096, 1024, 64, 64
N_CORES = 8
TQ = T // 2          # queries per core
ND = D_IN // 128     # contraction chunks (8)
NS = T // 128        # key chunks of 128 (32)
NP = T // 512        # x passes (8); passes 0-3 are the own query half
SCALE = float(1.0 / np.sqrt(np.float32(D_HEAD)))
LOG2E = 1.4426950408889634
EXP_A = SCALE * LOG2E * 128.0          # folds softmax scale into Schraudolph
EXP_B = 127.0 * 128.0 - 7.42

F32 = mybir.dt.float32
I16 = mybir.dt.int16
F32R = mybir.dt.float32r
BF16 = mybir.dt.bfloat16
EXPF = mybir.ActivationFunctionType.Exp
EXP_PATTERN = "AD"


def emit_body(nc, tc, io, dt_mm=None, phases="ABCD", n_iters=None):
    """Emit the per-core kernel body. io: dict of DRAM APs."""
    xt_d, wall_d, id_d = io["xt"], io["wall"], io["ident"]
    bkq_d, bkk_d, out_d = io["bkq"], io["bkk"], io["out"]

    with (
        tc.tile_pool(name="const", bufs=1) as cpool,
        tc.tile_pool(name="xt", bufs=6) as xpool,
        tc.tile_pool(name="proj", bufs=1) as ppool,
        tc.tile_pool(name="stg", bufs=2) as spool,
        tc.tile_pool(name="exp", bufs=28) as epool,
        tc.tile_pool(name="outp", bufs=1) as opool,
        tc.tile_pool(name="psum", bufs=1, space="PSUM") as qpool,
    ):
        # ---- constants: issue on the SP ring ahead of the xt streams so the
        # (serialized) DMA engines deliver them first ----
        wall_sb = cpool.tile([128, ND * 256], BF16)
        id_sb = cpool.tile([128, 64], BF16)
        bkq_sb = cpool.tile([128, 1], F32)
        bkk_sb = cpool.tile([64, 1], F32)
        ones_sb = cpool.tile([128, 1], BF16)
        nc.sync.dma_start(wall_sb[:, 0:1024], wall_d[:, 0:1024])
        nc.gpsimd.dma_start(bkq_sb[:], bkq_d[:])
        nc.gpsimd.dma_start(bkk_sb[:], bkk_d[:])
        nc.gpsimd.memset(ones_sb[:], 1.0)
        wkq_sb = wall_sb[:, 0:1024].rearrange("p (c m) -> p c m", m=128)
        wk_sb = wall_sb[:, 1024:1536].rearrange("p (c m) -> p c m", m=64)
        wv_sb = wall_sb[:, 1536:2048].rearrange("p (c m) -> p c m", m=64)

        # persistent per-iteration tensors
        kt = ppool.tile([64, T], BF16)             # k[h, s]
        qt = ppool.tile([64, TQ], BF16)            # q[h, t]
        v_sb = ppool.tile([128, NS, 64], BF16)     # v[s, o]
        pd_sb = ppool.tile([128, 2, 4, 8], F32)    # per-octet denom partials
        osb = opool.tile([128, 16, 64], F32)

        def body():
            xt_tiles = {}

            def load_xt(p, quarters=False):
                xt_t = xpool.tile([128, ND, 512], BF16, tag="xt", name=f"xt{p}")
                src = xt_d[:, p * 512:(p + 1) * 512]
                srcr = src.rearrange("(c p) t -> p c t", p=128)
                nh = 4 if quarters else 2
                for h in range(nh):
                    nc.sync.dma_start(
                        xt_t[:, h * (ND // nh):(h + 1) * (ND // nh), :],
                        srcr[:, h * (ND // nh):(h + 1) * (ND // nh), :])
                xt_tiles[p] = xt_t

            def own_pass(p):
                # k rows 0:64, q rows 64:128
                pkq = qpool.tile([128, 512], F32, tag="pa", bufs=2, name=f"pkq{p}")
                for d in range(ND):
                    nc.tensor.matmul(pkq[:], wkq_sb[:, d, :], xt_tiles[p][:, d, :],
                                     start=(d == 0), stop=(d == ND - 1))
                cols = slice(p * 512, (p + 1) * 512)
                stg = spool.tile([128, 512], BF16, tag="stg", name=f"stg{p}")
                nc.vector.tensor_scalar_add(kt[:, cols], pkq[0:64, :],
                                            bkq_sb[0:64])
                nc.vector.tensor_scalar_add(stg[64:128, :], pkq[64:128, :],
                                            bkq_sb[64:128])
                return stg

            def q_fix(p, stg):
                # shift q rows from partitions 64:128 down to 0:64 on the PE
                # (identity matmul) - avoids a DMA stuck behind the xt streams
                pqf = qpool.tile([64, 512], F32, tag="pa", bufs=2,
                                 name=f"pqf{p}")
                nc.tensor.matmul(pqf[:], id_sb[64:128, :], stg[64:128, :],
                                 start=True, stop=True)
                nc.vector.tensor_copy(qt[:, p * 512:(p + 1) * 512], pqf[:])

            def k_pass(p):
                pk = qpool.tile([64, 512], F32, tag="pa", bufs=2, name=f"pk{p}")
                for d in range(ND):
                    nc.tensor.matmul(pk[:], wk_sb[:, d, :], xt_tiles[p][:, d, :],
                                     start=(d == 0), stop=(d == ND - 1))
                cols = slice(p * 512, (p + 1) * 512)
                nc.vector.tensor_scalar_add(kt[:, cols], pk[:], bkk_sb[:])

            def v_pass(p):
                pv = qpool.tile([128, 4, 64], F32, tag="pa", bufs=2, name=f"pv{p}")
                for sc in range(4):
                    for d in range(ND):
                        nc.tensor.matmul(
                            pv[:, sc, :],
                            xt_tiles[p][:, d, sc * 128:(sc + 1) * 128],
                            wv_sb[:, d, :],
                            start=(sc == 0 and d == 0),
                            stop=(sc == 3 and d == ND - 1))
                nc.vector.tensor_copy(v_sb[:, p * 4:(p + 1) * 4, :], pv[:])

            pos = {}
            pds = {}
            ets = {}
            pd_done = {}
            pdps = {}


            def pd_octet(w, g):
                # denominator partial for windows s = 8g..8g+7: short-lived
                # psum tile rotating through the "pa" slots, evicted to SBUF
                pdp = qpool.tile([128, 8], F32, tag="pa", bufs=2,
                                 name=f"pdp{w}_{g}")
                for i, s in enumerate(range(8 * g, 8 * g + 8)):
                    for et, tc0, ntc in ets[(w, s)]:
                        for j in range(ntc):
                            tc = tc0 + j
                            nc.tensor.matmul(
                                pdp[:, tc:tc + 1],
                                et[:, j * 128:(j + 1) * 128],
                                ones_sb[:],
                                start=(i == 0 and tc == 0),
                                stop=(i == 7 and tc == 7))
                if g < 3:
                    nc.vector.tensor_copy(pd_sb[:, w, g, :], pdp[:])
                else:
                    pdps[w] = pdp
                pd_done[w] = pd_done.get(w, 0) + 1
                if pd_done[w] in (2, 3):
                    pd_presum(w)

            pending = []

            def av_mms(w, s, pi):
                po = pos[w]
                et, tc0, ntc = ets[(w, s)][pi]
                for j in range(ntc):
                    tc = tc0 + j
                    nc.tensor.matmul(po[:, tc, :],
                                     et[:, j * 128:(j + 1) * 128],
                                     v_sb[:, s, :],
                                     start=(s == 0 and tc == 0),
                                     stop=(s == NS - 1 and tc == 7))
                if s % 8 == 7 and tc0 + ntc == 8:
                    pd_octet(w, s // 8)

            def flush_pending():
                while pending:
                    av_mms(*pending.pop(0))

            def win(w, s, half=None):
                # half=None: full 1024-col window; half=0/1: 512-col window
                # covering query chunks tc 0..3 / 4..7 (used at startup while
                # only part of qt exists)
                ncols = 1024 if half is None else 512
                col0 = w * 1024 + (0 if half in (None, 0) else 512)
                tc0 = 0 if half in (None, 0) else 4
                ps = qpool.tile([128, ncols], F32, tag="ps", bufs=2,
                                name=f"ps{w}_{s}_{half}")
                for j in range(ncols // 512):
                    nc.tensor.matmul(
                        ps[:, j * 512:(j + 1) * 512],
                        kt[:, s * 128:(s + 1) * 128],
                        qt[:, col0 + j * 512:col0 + (j + 1) * 512],
                        start=True, stop=True)
                et = epool.tile([128, ncols], BF16, tag="et",
                                name=f"et{w}_{s}_{half}")
                idx = w * NS + s
                if half is None and idx % 3 == 2 and 6 < idx < 63:
                    gp_activation(et, ps, EXPF, SCALE)
                else:
                    nc.scalar.activation(et[:], ps[:], EXPF, scale=SCALE)
                ets.setdefault((w, s), []).append((et, tc0, ncols // 512 * 4))
                if w not in pos:
                    pos[w] = qpool.tile([128, 8, 64], F32, tag=f"po{w}", bufs=1,
                                        name=f"po{w}")
                # defer this window's AV until after the next window's scores,
                # so the activation latency is off the PE's in-order path
                flush_pending()
                pending.append((w, s, len(ets[(w, s)]) - 1))

            dsums = {}

            def pd_presum(w):
                # partial denominator sums emitted as octets complete
                dsum = dsums.setdefault(w, opool.tile(
                    [128, 2, 8], F32, tag="dsum", bufs=2, name=f"dsum{w}"))
                g = pd_done[w]
                if g == 2:
                    nc.vector.tensor_add(dsum[:, 0, :], pd_sb[:, w, 0, :],
                                         pd_sb[:, w, 1, :])
                elif g == 3:
                    nc.vector.tensor_add(dsum[:, 1, :], dsum[:, 0, :],
                                         pd_sb[:, w, 2, :])

            def finish(w):
                po = pos[w]
                dsum = dsums[w]
                rec = opool.tile([128, 8], F32, tag="rec", bufs=2,
                                 name=f"rec{w}")
                nc.vector.tensor_add(rec[:], dsum[:, 1, :], pdps[w][:])
                nc.vector.reciprocal(rec[:], rec[:])
                odst = out_d.rearrange("(j p) o -> p j o", p=128)
                for hh in range(2):
                    jj = w * 8 + hh * 4
                    nc.vector.tensor_mul(
                        osb[:, jj:jj + 4, :], po[:, hh * 4:(hh + 1) * 4, :],
                        rec[:, hh * 4:(hh + 1) * 4, None].broadcast_to(
                            [128, 4, 64]))
                    nc.sync.dma_start(odst[:, jj:jj + 4, :],
                                      osb[:, jj:jj + 4, :])

            # ---- emission schedule ----
            xt_tiles[0] = xpool.tile([128, ND, 512], BF16, tag="xt",
                                     name="xt0")
            xt_tiles[1] = xpool.tile([128, ND, 512], BF16, tag="xt",
                                     name="xt1")
            for p in range(2):
                srcr = xt_d[:, p * 512:(p + 1) * 512].rearrange(
                    "(c p) t -> p c t", p=128)
                for h in range(4):
                    nc.sync.dma_start(
                        xt_tiles[p][:, h * (ND // 4):(h + 1) * (ND // 4), :],
                        srcr[:, h * (ND // 4):(h + 1) * (ND // 4), :])
                if p == 0:
                    nc.sync.dma_start(wall_sb[:, 1024:2048],
                                      wall_d[:, 1024:2048])
                    nc.sync.dma_start(id_sb[:], id_d[:])
            stg0 = own_pass(0)
            v_pass(0)
            q_fix(0, stg0)
            # startup mini-windows: only qt cols 0:512 exist yet
            for s in range(0, 4):
                win(0, s, half=0)
            stg1 = own_pass(1)
            v_pass(1)
            q_fix(1, stg1)
            for s in range(0, 4):
                win(0, s, half=1)
            # passes 2-7 interleaved with windows; once qt is complete
            # (pass 3) also consume w=1 so windows never outrun production
            done1 = 0
            for p in range(2, NP):
                load_xt(p, quarters=(p < 4))
                if p < 4:
                    stgp = own_pass(p)
                    v_pass(p)
                    q_fix(p, stgp)
                else:
                    k_pass(p)
                    v_pass(p)
                for s in range(4 * (p - 2) + 4, 4 * (p - 2) + 8):
                    win(0, s)
                    if p >= 4:
                        win(1, done1)
                        done1 += 1
            for s in range(28, NS):
                win(0, s)
                win(1, done1)
                done1 += 1
            first_tail = done1
            for s in range(first_tail, NS):
                win(1, s)
                if s == first_tail + 1:
                    flush_w0 = [p_ for p_ in pending if p_[0] == 0]
                    for p_ in flush_w0:
                        pending.remove(p_)
                        av_mms(*p_)
                    finish(0)
            flush_pending()
            finish(1)

        if n_iters is None:
            body()
        else:
            with tc.For_i(0, n_iters, 1) as _i:
                body()


def build_program(dt_mm=None, phases="ABCD", n_iters=None):
    nc = bacc.Bacc("TRN2", target_bir_lowering=False, debug=False,
                   num_devices=N_CORES)
    io = {
        "xt": nc.dram_tensor("xt", [D_IN, T], BF16, kind="ExternalInput").ap(),
        "wall": nc.dram_tensor("wall", [128, ND * 256], BF16, kind="ExternalInput").ap(),
        "ident": nc.dram_tensor("ident", [128, 64], BF16, kind="ExternalInput").ap(),
        "bkq": nc.dram_tensor("bkq", [128, 1], F32, kind="ExternalInput").ap(),
        "bkk": nc.dram_tensor("bkk", [64, 1], F32, kind="ExternalInput").ap(),
        "out": nc.dram_tensor("out", [TQ, D_OUT], F32, kind="ExternalOutput").ap(),
    }
    with tile.TileContext(nc) as tc:
        emit_body(nc, tc, io, dt_mm, phases=phases, n_iters=n_iters)
    nc.compile()
    return nc


_PROGRAM_CACHE = {}


def get_program(dt_mm=None):
    key = str(dt_mm)
    if key not in _PROGRAM_CACHE:
        _PROGRAM_CACHE[key] = build_program(dt_mm)
    return _PROGRAM_CACHE[key]


def make_in_maps(x, Wk, bk, Wq, bq, Wv, bv):
    bf = ml_dtypes.bfloat16
    x = np.asarray(x, dtype=np.float32)
    # pack [p, cols]: wkq block (8 chunks x 128) | wk (8 x 64) | wv (8 x 64)
    def blk(Wm):
        m = Wm.shape[1]
        return Wm.astype(np.float32).reshape(ND, 128, m).transpose(1, 0, 2) \
            .reshape(128, ND * m)
    wall = np.ascontiguousarray(np.concatenate(
        [blk(np.concatenate([Wk, Wq], axis=1)), blk(Wk), blk(Wv)],
        axis=1)).astype(bf)
    ident = np.zeros((128, 64), dtype=np.float32)
    ident[64:128, :] = np.eye(64)
    ident = ident.astype(bf)
    bkq = np.concatenate([bk, bq]).astype(np.float32).reshape(128, 1)
    bkk = np.asarray(bk, dtype=np.float32).reshape(64, 1)
    in_maps = []
    for c in range(N_CORES):
        b, half = c // 2, c % 2
        xb = x[b]
        own = xb[half * TQ:(half + 1) * TQ].T
        other = xb[(1 - half) * TQ:(2 - half) * TQ].T
        xt = np.ascontiguousarray(
            np.concatenate([own, other], axis=1)).astype(bf)
        in_maps.append({"xt": xt, "wall": wall, "ident": ident,
                       "bkq": bkq, "bkk": bkk})
    return in_maps


def assemble(results, bv):
    out = np.empty((B, T, D_OUT), dtype=np.float32)
    bv = np.asarray(bv, dtype=np.float32)
    for c in range(N_CORES):
        b, half = c // 2, c % 2
        out[b, half * TQ:(half + 1) * TQ, :] = results[c]["out"] + bv
    return out


def kernel(x, Wk, bk, Wq, bq, Wv, bv):
    nc = get_program()
    in_maps = make_in_maps(x, Wk, bk, Wq, bq, Wv, bv)
    res = run_bass_kernel_spmd(nc, in_maps, list(range(N_CORES)))
    return assemble(res.results, bv)
